# revision 1
# baseline (speedup 1.0000x reference)
"""Trainium2 Bass kernel for 2-layer GAT (nn_GAT_45157286150549).

8-core SPMD: core c owns destination nodes [c*6272, (c+1)*6272).
Layer-1/2 edge aggregation via sorted-by-dst edges + one-hot selection-matrix
matmuls; source-row gathers via the custom SWDGE dma_gather instruction.
Softmax is post-normalized: out = (sum_e ex_e * h_src_e) / (sum_e ex_e).
"""

import os
import sys

for _p in ("/opt/trn_rl_repo", "/root/.axon_site/_ro/trn_rl_repo"):
    if os.path.isdir(_p) and _p not in sys.path:
        sys.path.insert(0, _p)

import numpy as np
import ml_dtypes

import concourse.bass as bass
import concourse.bacc as bacc
import concourse.mybir as mybir
import concourse.tile as tile
from concourse.library_config import mlp
from concourse.tile import add_dep_helper
from concourse.bass_utils import run_bass_kernel_spmd

# ---------------- problem constants ----------------
N, F_IN, E = 50000, 128, 800000
HID, HEADS, EMB = 32, 8, 64
NEG_SLOPE = 0.2

NCORES = 8
P = 128
NB = 49                    # node blocks per core
NODES_PC = NB * P          # 6272
NTOT = NCORES * NODES_PC   # 50176
LO_ROWS = 32768            # int16 gather-index split
HI_ROWS = NTOT - LO_ROWS   # 17408
LO_BLKS = LO_ROWS // P     # 256
ALL_BLKS = NTOT // P       # 392

T1_W_F32, T1_W_BF16 = 320, 384  # layer-1 table row (264 used, 256B-aligned)
T2_W = 128                 # layer-2 table row width (65 used)

F32 = mybir.dt.float32
F32R = mybir.dt.float32r
BF16 = mybir.dt.bfloat16
I16 = mybir.dt.int16

# matmul dtype knob: "f32" (exact, 4 cyc/row) or "f32r" (1 cyc/row at N>=256)
MM_DT = os.environ.get("GAT_MM_DT", "f32r")


BF_MODE = MM_DT == "bf16"
MDT = F32R if MM_DT in ("f32r", "bf16") else F32   # phase-A matmul dtype
PDT = BF16 if BF_MODE else MDT                     # layer-1 P/exm/table chain
TDT = BF16 if BF_MODE else F32                     # layer-1 gather table dtype


# ============================================================
# Device program
# ============================================================

PHASES = os.environ.get("GAT_PHASES", "abgc")  # bisection: a, ab, abg, abgc
SB_BUFS = int(os.environ.get("GAT_SB_BUFS", "10"))
G_BUFS = int(os.environ.get("GAT_G_BUFS", "4"))
PTP_BUFS = int(os.environ.get("GAT_PTP_BUFS", "3"))
ADP_BUFS = int(os.environ.get("GAT_ADP_BUFS", "2"))
LRELU_ACT = os.environ.get("GAT_LRELU", "dve") == "act"
AGG_BUFS = int(os.environ.get("GAT_AGG_BUFS", "1"))
PA_BATCH = int(os.environ.get("GAT_PA_BATCH", "8"))


def build_nc(nlo_ch: int, nhi_ch: int, nlo_b=None, nhi_b=None):
    T1_W = T1_W_BF16 if BF_MODE else T1_W_F32
    nch = nlo_ch + nhi_ch
    if nlo_b is None:
        nlo_b = [nlo_ch] * NB
    if nhi_b is None:
        nhi_b = [nhi_ch] * NB
    nc = bacc.Bacc("TRN2", target_bir_lowering=False, debug=False,
                   num_devices=NCORES)

    dt = nc.dram_tensor
    xTg = dt("xTg", [F_IN, NTOT], MDT, kind="ExternalInput").ap()
    xTo = dt("xTo", [F_IN, NODES_PC], MDT, kind="ExternalInput").ap()
    W1cat = dt("W1cat", [F_IN, 264], MDT, kind="ExternalInput").ap()
    Ad1 = dt("Ad1", [F_IN, 8], MDT, kind="ExternalInput").ap()
    W2cat = dt("W2cat", [2, P, 66], F32, kind="ExternalInput").ap()
    b1t = dt("b1t", [P, 256], F32, kind="ExternalInput").ap()
    b2t = dt("b2t", [P, 64], F32, kind="ExternalInput").ap()
    iotam = dt("iotam", [P, P], PDT if BF_MODE else F32, kind="ExternalInput").ap()
    identm = dt("identm", [P, P], PDT, kind="ExternalInput").ap()
    identmf = dt("identmf", [P, P], F32, kind="ExternalInput").ap()
    idx_lo = dt("idx_lo", [NB, P, nlo_ch * 8], I16, kind="ExternalInput").ap()
    idx_hi = dt("idx_hi", [NB, P, nhi_ch * 8], I16, kind="ExternalInput").ap()
    dstl = dt("dstl", [NB, P, nch], F32, kind="ExternalInput").ap()

    out2 = dt("out2", [NODES_PC, EMB], F32, kind="ExternalOutput").ap()

    hA_lo = dt("hA_lo", [LO_ROWS, T1_W], TDT).ap()
    hA_hi = dt("hA_hi", [HI_ROWS, T1_W], TDT).ap()
    cc_in = dt("cc_in", [NODES_PC, T2_W], F32).ap()
    h2A = dt("h2A", [NTOT, T2_W], F32, addr_space="Shared").ap()

    with tile.TileContext(nc) as tc:
        with (
            tc.tile_pool(name="const", bufs=1) as cp,
            tc.tile_pool(name="persist", bufs=1) as pp,
        ):
            lib_inst = nc.gpsimd.load_library(mlp)

            def gather(**kw):
                g = nc.gpsimd.dma_gather(**kw)
                add_dep_helper(g.ins, lib_inst.ins, sync=True,
                               reason="mlp library before gather")
                return g

            def gather_split(g3, table, idx_tile, n_ch, ch_off, elem):
                # dma_gather is limited to ~1024 indices per call
                for off in range(0, n_ch, 8):
                    k = min(8, n_ch - off)
                    gather(
                        out_ap=g3[:, ch_off + off:ch_off + off + k, :],
                        in_ap=table,
                        idxs_ap=idx_tile[:, off * 8:(off + k) * 8],
                        num_idxs=k * P, num_idxs_reg=k * P, elem_size=elem)
            w1_sb = cp.tile([F_IN, 264], MDT)
            nc.sync.dma_start(out=w1_sb[:], in_=W1cat[:])
            ad1_sb = cp.tile([F_IN, 8], MDT)
            nc.sync.dma_start(out=ad1_sb[:], in_=Ad1[:])
            w2_sb0 = cp.tile([P, 66], F32, tag="w2a")
            nc.sync.dma_start(out=w2_sb0[:], in_=W2cat[0])
            w2_sb1 = cp.tile([P, 66], F32, tag="w2b")
            nc.sync.dma_start(out=w2_sb1[:], in_=W2cat[1])
            b1_sb = cp.tile([P, 256], F32)
            nc.sync.dma_start(out=b1_sb[:], in_=b1t[:])
            b2_sb = cp.tile([P, 64], F32)
            nc.sync.dma_start(out=b2_sb[:], in_=b2t[:])
            iota_sb = cp.tile([P, P], PDT if BF_MODE else F32)
            nc.sync.dma_start(out=iota_sb[:], in_=iotam[:])
            ident_sb = cp.tile([P, P], PDT)
            nc.sync.dma_start(out=ident_sb[:], in_=identm[:])
            identf_sb = cp.tile([P, P], F32, tag="identf")
            nc.sync.dma_start(out=identf_sb[:], in_=identmf[:])

            a_dst1 = pp.tile([P, NB * 8], PDT)   # per-block a_dst (layer 1)
            a_dst2 = pp.tile([P, NB], F32)       # per-block a_dst (layer 2)

            # ---------- Phase A: hA table for all nodes ----------
            with (
                tc.tile_pool(name="pa_sb", bufs=3) as pa,
                tc.tile_pool(name="pa_ps", bufs=2, space="PSUM") as paps,
            ):
                assert LO_BLKS % PA_BATCH == 0 and ALL_BLKS % PA_BATCH == 0
                for blk0 in range(0, ALL_BLKS, PA_BATCH):
                    xt = pa.tile([F_IN, PA_BATCH * P], MDT, tag="xt")
                    nc.sync.dma_start(
                        out=xt[:], in_=xTg[:, blk0 * P:(blk0 + PA_BATCH) * P])
                    hs = pa.tile([P, PA_BATCH * 264], TDT, tag="hs")
                    for k in range(PA_BATCH):
                        ps = paps.tile([P, 264], F32, tag="ps")
                        nc.tensor.matmul(
                            ps[:], lhsT=xt[:, k * P:(k + 1) * P],
                            rhs=w1_sb[:], start=True, stop=True)
                        nc.any.tensor_copy(
                            out=hs[:, k * 264:(k + 1) * 264], in_=ps[:])
                    tab = hA_lo if blk0 < LO_BLKS else hA_hi
                    r0 = blk0 * P if blk0 < LO_BLKS else (blk0 - LO_BLKS) * P
                    dst_rows = bass.AP(
                        tab.tensor, r0 * T1_W,
                        [[T1_W, P], [T1_W * P, PA_BATCH], [1, 264]])
                    nc.sync.dma_start(out=dst_rows, in_=hs[:].rearrange(
                        "p (k e) -> p k e", e=264))

                # Phase A2: a_dst1 for owned nodes
                for b in range(NB):
                    xt = pa.tile([F_IN, P], MDT, tag="xt2")
                    nc.sync.dma_start(
                        out=xt[:], in_=xTo[:, b * P:(b + 1) * P])
                    ps8 = paps.tile([P, 8], F32, tag="ps8")
                    nc.tensor.matmul(ps8[:], lhsT=xt[:], rhs=ad1_sb[:],
                                     start=True, stop=True)
                    nc.vector.tensor_copy(
                        out=a_dst1[:, b * 8:(b + 1) * 8], in_=ps8[:])

            # ---------- Phase B: layer-1 edge pass + layer-2 node compute ----
            with (
                tc.tile_pool(name="pb_g", bufs=G_BUFS) as pg,
                tc.tile_pool(name="pb_sb", bufs=SB_BUFS) as pb,
                tc.tile_pool(name="pb_ptp", bufs=PTP_BUFS, space="PSUM") as pbps,
                tc.tile_pool(name="pb_adp", bufs=ADP_BUFS, space="PSUM") as pbadp,
                tc.tile_pool(name="pb_l2", bufs=1, space="PSUM") as pbl2,
                tc.tile_pool(name="pb_ps1", bufs=AGG_BUFS, space="PSUM") as pbps1,
            ):
                for b in range(NB if "b" in PHASES else 0):
                    dl = pb.tile([P, nch], F32, tag="dl")
                    nc.sync.dma_start(out=dl[:], in_=dstl[b])
                    il = pb.tile([P, nlo_ch * 8], I16, tag="il")
                    nc.sync.dma_start(out=il[:], in_=idx_lo[b])
                    ih = pb.tile([P, nhi_ch * 8], I16, tag="ih")
                    nc.sync.dma_start(out=ih[:], in_=idx_hi[b])

                    G = pg.tile([P, nch * T1_W], TDT, tag="G")
                    g3 = G[:].rearrange("p (c e) -> p c e", e=T1_W)
                    nch_b = nlo_b[b] + nhi_b[b]
                    gather_split(g3, hA_lo[:], il, nlo_b[b], 0, T1_W)
                    gather_split(g3, hA_hi[:], ih, nhi_b[b], nlo_b[b], T1_W)

                    agg = pbps1.tile([P, 264], F32, tag="agg")
                    for j in range(nch_b):
                        gh = G[:, j * T1_W:j * T1_W + 256]
                        ga = G[:, j * T1_W + 256:j * T1_W + 264]
                        Pm = pb.tile([P, P], PDT, tag="Pm")
                        nc.vector.tensor_scalar(
                            out=Pm[:], in0=iota_sb[:],
                            scalar1=dl[:, j:j + 1], scalar2=None,
                            op0=mybir.AluOpType.is_equal)
                        ptp = pbps.tile([P, P], PDT, tag="ptp")
                        nc.tensor.transpose(
                            out=ptp[:], in_=Pm[:], identity=ident_sb[:])
                        pt = pb.tile([P, P], PDT, tag="pt")
                        nc.scalar.copy(out=pt[:], in_=ptp[:])
                        adp = pbadp.tile([P, 8], F32, tag="adp")
                        nc.tensor.matmul(
                            adp[:], lhsT=pt[:],
                            rhs=a_dst1[:, b * 8:(b + 1) * 8],
                            start=True, stop=True)
                        ee = pb.tile([P, 8], F32, tag="ee")
                        nc.vector.tensor_add(out=ee[:], in0=ga, in1=adp[:])
                        # leaky_relu = max(x, 0.2x), then exp
                        e2 = pb.tile([P, 8], F32, tag="e2")
                        if LRELU_ACT:
                            nc.scalar.activation(
                                out=e2[:], in_=ee[:],
                                func=mybir.ActivationFunctionType.Lrelu,
                                alpha=NEG_SLOPE)
                        elif os.environ.get("GAT_LRELU", "dve") == "pool":
                            nc.gpsimd.tensor_scalar_mul(e2[:], ee[:], NEG_SLOPE)
                            nc.gpsimd.tensor_tensor(
                                out=e2[:], in0=ee[:], in1=e2[:],
                                op=mybir.AluOpType.max)
                        else:
                            nc.vector.tensor_scalar_mul(e2[:], ee[:], NEG_SLOPE)
                            nc.vector.tensor_tensor(
                                out=e2[:], in0=ee[:], in1=e2[:],
                                op=mybir.AluOpType.max)
                        exm = pb.tile([P, 264], PDT, tag="exm")
                        nc.scalar.activation(
                            out=exm[:, 0:8], in_=e2[:],
                            func=mybir.ActivationFunctionType.Exp)
                        nc.vector.tensor_tensor(
                            out=exm[:, 8:264].rearrange(
                                "p (h c) -> p h c", c=HID),
                            in0=gh.rearrange("p (h c) -> p h c", c=HID),
                            in1=(exm[:, 0:8] if BF_MODE else
                                 exm[:, 0:8].bitcast(F32)).to_broadcast(
                                [P, 8, HID]),
                            op=mybir.AluOpType.mult)
                        nc.tensor.matmul(
                            agg[:], lhsT=Pm[:], rhs=exm[:],
                            start=(j == 0), stop=(j == nch_b - 1))

                    den = pb.tile([P, 8], F32, tag="den")
                    nc.vector.tensor_scalar_add(den[:], agg[:, 0:8], 1e-16)
                    R = pb.tile([P, 8], F32, tag="R")
                    nc.vector.reciprocal(R[:], den[:])
                    h1 = pb.tile([P, 256], F32, tag="h1")
                    nc.vector.tensor_tensor(
                        out=h1[:].rearrange("p (h c) -> p h c", c=HID),
                        in0=agg[:, 8:264].rearrange("p (h c) -> p h c", c=HID),
                        in1=R[:].to_broadcast([P, 8, HID]),
                        op=mybir.AluOpType.mult)
                    nc.vector.tensor_add(out=h1[:], in0=h1[:], in1=b1_sb[:])
                    nc.vector.tensor_scalar_max(h1[:], h1[:], 0.0)

                    # layer-2 node compute for this block
                    ps2 = pbl2.tile([P, 66], F32, tag="ps2")
                    for k in range(2):
                        tp = pbl2.tile([P, P], F32, tag="tp")
                        nc.tensor.transpose(
                            out=tp[:], in_=h1[:, k * P:(k + 1) * P],
                            identity=identf_sb[:])
                        ts = pb.tile([P, P], F32, tag="ts")
                        nc.scalar.copy(out=ts[:], in_=tp[:])
                        nc.tensor.matmul(
                            ps2[:], lhsT=ts[:],
                            rhs=(w2_sb0[:] if k == 0 else w2_sb1[:]),
                            start=(k == 0), stop=(k == 1))
                    h2 = pb.tile([P, 65], F32, tag="h2")
                    nc.scalar.copy(out=h2[:], in_=ps2[:, 0:65])
                    nc.vector.tensor_copy(
                        out=a_dst2[:, b:b + 1], in_=ps2[:, 65:66])
                    nc.sync.dma_start(
                        out=cc_in[b * P:(b + 1) * P, 0:65], in_=h2[:])

            # ---------- AllGather layer-2 table ----------
            if "g" in PHASES:
                nc.gpsimd.collective_compute(
                "AllGather", mybir.AluOpType.bypass,
                    replica_groups=[list(range(NCORES))],
                    ins=[cc_in[:].opt()], outs=[h2A[:].opt()])

            # ---------- Phase C: layer-2 edge pass ----------
            with (
                tc.tile_pool(name="pc_g", bufs=G_BUFS) as pg2,
                tc.tile_pool(name="pc_sb", bufs=SB_BUFS) as pc,
                tc.tile_pool(name="pc_ptp", bufs=PTP_BUFS, space="PSUM") as pcps,
                tc.tile_pool(name="pc_adp", bufs=ADP_BUFS, space="PSUM") as pcadp,
                tc.tile_pool(name="pc_ps1", bufs=AGG_BUFS, space="PSUM") as pcps1,
            ):
                for b in range(NB if "c" in PHASES else 0):
                    dl = pc.tile([P, nch], F32, tag="dl")
                    nc.sync.dma_start(out=dl[:], in_=dstl[b])
                    il = pc.tile([P, nlo_ch * 8], I16, tag="il")
                    nc.sync.dma_start(out=il[:], in_=idx_lo[b])
                    ih = pc.tile([P, nhi_ch * 8], I16, tag="ih")
                    nc.sync.dma_start(out=ih[:], in_=idx_hi[b])

                    G2 = pg2.tile([P, nch * T2_W], F32, tag="G2")
                    g3 = G2[:].rearrange("p (c e) -> p c e", e=T2_W)
                    nch_b = nlo_b[b] + nhi_b[b]
                    gather_split(g3, h2A[0:LO_ROWS, :], il, nlo_b[b], 0, T2_W)
                    gather_split(g3, h2A[LO_ROWS:NTOT, :], ih, nhi_b[b],
                                 nlo_b[b], T2_W)

                    agg = pcps1.tile([P, 65], F32, tag="agg2")
                    for j in range(nch_b):
                        gh = G2[:, j * T2_W:j * T2_W + 64]
                        ga = G2[:, j * T2_W + 64:j * T2_W + 65]
                        Pm = pc.tile([P, P], F32, tag="Pm")
                        nc.vector.tensor_scalar(
                            out=Pm[:], in0=iota_sb[:],
                            scalar1=dl[:, j:j + 1], scalar2=None,
                            op0=mybir.AluOpType.is_equal)
                        ptp = pcps.tile([P, P], F32, tag="ptp")
                        nc.tensor.transpose(
                            out=ptp[:], in_=Pm[:], identity=identf_sb[:])
                        pt = pc.tile([P, P], F32, tag="pt")
                        nc.scalar.copy(out=pt[:], in_=ptp[:])
                        adp = pcadp.tile([P, 1], F32, tag="adp")
                        nc.tensor.matmul(
                            adp[:], lhsT=pt[:], rhs=a_dst2[:, b:b + 1],
                            start=True, stop=True)
                        ee = pc.tile([P, 1], F32, tag="ee")
                        nc.vector.tensor_add(out=ee[:], in0=ga, in1=adp[:])
                        e2 = pc.tile([P, 1], F32, tag="e2")
                        nc.vector.tensor_scalar_mul(e2[:], ee[:], NEG_SLOPE)
                        nc.vector.tensor_tensor(
                            out=e2[:], in0=ee[:], in1=e2[:],
                            op=mybir.AluOpType.max)
                        exm = pc.tile([P, 65], F32, tag="exm")
                        nc.scalar.activation(
                            out=exm[:, 0:1], in_=e2[:],
                            func=mybir.ActivationFunctionType.Exp)
                        nc.vector.tensor_scalar(
                            out=exm[:, 1:65], in0=gh,
                            scalar1=exm[:, 0:1], scalar2=None,
                            op0=mybir.AluOpType.mult)
                        nc.tensor.matmul(
                            agg[:], lhsT=Pm[:], rhs=exm[:],
                            start=(j == 0), stop=(j == nch_b - 1))

                    den = pc.tile([P, 1], F32, tag="den")
                    nc.vector.tensor_scalar_add(den[:], agg[:, 0:1], 1e-16)
                    R = pc.tile([P, 1], F32, tag="R")
                    nc.vector.reciprocal(R[:], den[:])
                    o2 = pc.tile([P, 64], F32, tag="o2")
                    nc.vector.tensor_scalar(
                        out=o2[:], in0=agg[:, 1:65],
                        scalar1=R[:], scalar2=None,
                        op0=mybir.AluOpType.mult)
                    nc.vector.tensor_add(out=o2[:], in0=o2[:], in1=b2_sb[:])
                    nc.sync.dma_start(
                        out=out2[b * P:(b + 1) * P, :], in_=o2[:])

    nc.compile()
    return nc


# ============================================================
# Host preprocessing
# ============================================================

def _idx_stream(flat_i16: np.ndarray) -> np.ndarray:
    """[L] int16 -> [128, L//16]: element (p, s) = flat[s*16 + p%16]."""
    L = len(flat_i16)
    a16 = flat_i16.reshape(L // 16, 16).T          # [16, L//16]
    return np.tile(a16, (8, 1)).astype(np.int16)   # [128, L//16]


def prepare(x, edge_index, W_src1, W_dst1, att_src1, att_dst1, b1,
            W_src2, W_dst2, att_src2, att_dst2, b2):
    x = np.asarray(x, np.float32)
    src = np.asarray(edge_index[0], np.int64)
    dst = np.asarray(edge_index[1], np.int64)

    # weight folding: a = x @ (W @ blockdiag(att))
    att1s = np.asarray(att_src1, np.float32)  # [H, C]
    att1d = np.asarray(att_dst1, np.float32)
    bd1s = np.zeros((HEADS * HID, HEADS), np.float32)
    bd1d = np.zeros((HEADS * HID, HEADS), np.float32)
    for h in range(HEADS):
        bd1s[h * HID:(h + 1) * HID, h] = att1s[h]
        bd1d[h * HID:(h + 1) * HID, h] = att1d[h]
    A_src1 = np.asarray(W_src1, np.float32) @ bd1s     # [128, 8]
    A_dst1 = np.asarray(W_dst1, np.float32) @ bd1d     # [128, 8]
    W1cat = np.concatenate([np.asarray(W_src1, np.float32), A_src1], axis=1)

    A_src2 = np.asarray(W_src2, np.float32) @ np.asarray(att_src2, np.float32).reshape(EMB, 1)
    A_dst2 = np.asarray(W_dst2, np.float32) @ np.asarray(att_dst2, np.float32).reshape(EMB, 1)
    W2cat = np.concatenate(
        [np.asarray(W_src2, np.float32), A_src2, A_dst2], axis=1)  # [256, 66]
    W2cat = W2cat.reshape(2, P, 66)

    xTg = np.zeros((F_IN, NTOT), np.float32)
    xTg[:, :N] = x.T

    # ---- edge sharding ----
    owner = dst // NODES_PC
    per_core = []
    max_lo_ch = 1
    max_hi_ch = 1
    for c in range(NCORES):
        m = owner == c
        s_c, d_c = src[m], dst[m]
        dloc = d_c - c * NODES_PC
        blk = dloc // P
        blocks = []
        for b in range(NB):
            mb_ = blk == b
            sb_, db_ = s_c[mb_], dloc[mb_] - b * P
            lo = sb_ < LO_ROWS
            s_lo, d_lo = sb_[lo], db_[lo]
            s_hi, d_hi = sb_[~lo] - LO_ROWS, db_[~lo]
            blocks.append((s_lo, d_lo, s_hi, d_hi))
            max_lo_ch = max(max_lo_ch, (len(s_lo) + P - 1) // P)
            max_hi_ch = max(max_hi_ch, (len(s_hi) + P - 1) // P)
        per_core.append(blocks)

    nlo_ch, nhi_ch = max_lo_ch, max_hi_ch
    nch = nlo_ch + nhi_ch
    nlo_b = [max((len(per_core[c][b][0]) + P - 1) // P or 1
                 for c in range(NCORES)) for b in range(NB)]
    nhi_b = [max((len(per_core[c][b][2]) + P - 1) // P or 1
                 for c in range(NCORES)) for b in range(NB)]
    nlo_b = [max(v, 1) for v in nlo_b]
    nhi_b = [max(v, 1) for v in nhi_b]

    def pad_to(a, L, fill):
        out = np.full(L, fill, a.dtype)
        out[:len(a)] = a
        return out

    pdt_np = ml_dtypes.bfloat16 if BF_MODE else np.float32
    in_maps = []
    common = {
        "xTg": xTg,
        "W1cat": W1cat, "Ad1": A_dst1, "W2cat": W2cat,
        "b1t": np.tile(np.asarray(b1, np.float32)[None, :], (P, 1)),
        "b2t": np.tile(np.asarray(b2, np.float32)[None, :], (P, 1)),
        "iotam": np.tile(np.arange(P, dtype=np.float32)[None, :],
                         (P, 1)).astype(pdt_np),
        "identm": np.eye(P, dtype=np.float32).astype(
            pdt_np if BF_MODE else np.float32),
        "identmf": np.eye(P, dtype=np.float32),
    }
    for c in range(NCORES):
        idx_lo_c = np.zeros((NB, P, nlo_ch * 8), np.int16)
        idx_hi_c = np.zeros((NB, P, nhi_ch * 8), np.int16)
        dstl_c = np.full((NB, P, nch), 999.0, np.float32)  # cast at end
        for b in range(NB):
            s_lo, d_lo, s_hi, d_hi = per_core[c][b]
            nl, nh = nlo_b[b], nhi_b[b]
            slo = pad_to(s_lo.astype(np.int16), nlo_ch * P, 0)
            shi = pad_to(s_hi.astype(np.int16), nhi_ch * P, 0)
            idx_lo_c[b] = _idx_stream(slo)
            idx_hi_c[b] = _idx_stream(shi)
            dl_lo = pad_to(d_lo.astype(np.float32), nl * P, 999.0)
            dl_hi = pad_to(d_hi.astype(np.float32), nh * P, 999.0)
            dstl_c[b, :, 0:nl] = dl_lo.reshape(nl, P).T
            dstl_c[b, :, nl:nl + nh] = dl_hi.reshape(nh, P).T
        in_maps.append({
            **common,
            "xTo": xTg[:, c * NODES_PC:(c + 1) * NODES_PC].copy(),
            "idx_lo": idx_lo_c, "idx_hi": idx_hi_c,
            "dstl": dstl_c,
        })
    return in_maps, nlo_ch, nhi_ch, tuple(nlo_b), tuple(nhi_b)


_NC_CACHE = {}


def kernel(**inputs) -> np.ndarray:
    in_maps, nlo_ch, nhi_ch, nlo_b, nhi_b = prepare(**inputs)
    key = (nlo_ch, nhi_ch, nlo_b, nhi_b, MM_DT, PHASES)
    if key not in _NC_CACHE:
        _NC_CACHE[key] = build_nc(nlo_ch, nhi_ch, nlo_b, nhi_b)
    nc = _NC_CACHE[key]
    res = run_bass_kernel_spmd(
        nc, in_maps, core_ids=list(range(NCORES)),
        trace=bool(int(os.environ.get("GAT_TRACE", "0"))))
    kernel.last_results = res
    out = np.concatenate(
        [res.results[c]["out2"] for c in range(NCORES)], axis=0)
    return out[:N].astype(np.float32)

